# revision 19
# baseline (speedup 1.0000x reference)
"""Trainium2 Bass kernel for nn_MCUDetectionLoss.

Strategy (data-parallel over batch, 8 cores, B=16 -> 2 images/core):

The loss touches (a) the objectness channel cls_p[:, 0] in full and (b) 64
gathered cells per image (obj + 63-class column + 4 reg values).  The host
ships each core:
  - obj  [128, 320]  objectness maps (scale3 flat 32768 = cols 0:256,
                     scale4 flat 8192 = cols 256:320)
  - meta [128, 140]  one row per target: gathered prediction values at the
                     target cell plus pure-index metadata (one-hot class,
                     box-offset constants, duplicate-cell weights, masks)

Device program per core: softplus of the gathered logits via one exp/ln
pass (ACT table set 6: exp+ln+copy, one table load), sigmoids via DVE
reciprocal of 1+e^x, smooth-L1 box loss, positive BCE, focal loss,
softplus-sum of the full obj map, and a [128,2]x[128,6] matmul reducing
everything to per-scale partial sums.  The host combines the 8 [2,6]
partials into the scalar.

Identities used (bce = BCEWithLogits):
  bce(x, 0) = softplus(x);  bce(x, 1) = softplus(x) - x
  sigmoid(x) = 1/(1 + exp(-x));  1 - sigmoid(x) = 1/(1 + exp(x))
  focal (1-pt)^2 = (y-p)^2
  exp(clip(x,-4,4)) = clip(exp(x), e^-4, e^4)
  sum softplus(obj)*bg = sum_all - sum_targets softplus(obj_t)/cnt_t
  smooth_l1(d) = mt*(|d| - mt/2),  mt = min(|d|, 1)
where cnt_t = multiplicity of the target's (image, cell) -- precomputed on
host from the integer cell indices (pure metadata, no tensor values).
"""

import math
import sys

for _p in ("/opt/trn_rl_repo", "/root/.axon_site/_ro/trn_rl_repo"):
    if _p not in sys.path:
        sys.path.append(_p)

import ml_dtypes
import numpy as np

import concourse.bass as bass
from concourse import mybir
from concourse.bass_utils import run_bass_kernel_spmd

AF = mybir.ActivationFunctionType
ALU = mybir.AluOpType
AX = mybir.AxisListType
F32 = mybir.dt.float32
BF16 = mybir.dt.bfloat16

ALPHA = 0.25
BBOX_W, OBJ_W, CLS_W = 2.0, 1.0, 0.5

M = 8          # cores
B, T, CC = 16, 32, 63
H3 = W3 = 128
H4 = W4 = 64
BL = B // M    # images per core
NT = 2 * BL * T     # 128 targets per core (rows 0:64 scale3, 64:128 scale4)
OBJW = (BL * H3 * W3 + BL * H4 * W4) // 128   # 320
C3 = BL * H3 * W3 // 128                      # 256 obj cols of scale3

# meta column layout
O_, Z_, NR_, R23_, Y_ = 0, 1, 64, 66, 68
A_, REC_, MK_, ONE_ = 131, 135, 136, 138
CM = 139

E4 = float(math.exp(4.0))
EN4 = float(math.exp(-4.0))

_NC_CACHE = None


def _build_bass():
    nc = bass.Bass("TRN2", target_bir_lowering=False, debug=False, num_devices=M)
    obj = nc.declare_dram_parameter("obj", [128, OBJW], BF16, isOutput=False)
    meta = nc.declare_dram_parameter("meta", [NT, CM], F32, isOutput=False)
    part = nc.declare_dram_parameter("part", [NT, 7], F32, isOutput=True)

    from contextlib import ExitStack
    with ExitStack() as st:
        def sb(name, shape, dt=F32):
            return st.enter_context(nc.sbuf_tensor(name, shape, dt))

        meta_t = sb("meta_t", [NT, CM]); obj_t = sb("obj_t", [128, OBJW], BF16)
        ebuf = sb("ebuf", [NT, 68]); sclb = sb("sclb", [NT, 66])
        rxb = sb("rxb", [NT, 65])          # 0:63 = 1-p(z), 63:65 = sig(r01)
        eob = sb("eob", [128, OBJW]); spb = sb("spb", [128, OBJW])
        dwh = sb("dwh", [NT, 2])
        pq = sb("pq", [NT, 4]); dt_ = sb("dt_", [NT, 4]); dab = sb("dab", [NT, 4])
        mt = sb("mt", [NT, 4]); rl = sb("rl", [NT, 4])
        sl1o = sb("sl1o", [NT, 4]); rlo = sb("rlo", [NT, 4])
        xy = sb("xy", [NT, CC]); u1 = sb("u1", [NT, CC])
        bce = sb("bce", [NT, CC]); q2 = sb("q2", [NT, CC]); fqo = sb("fqo", [NT, CC])
        stats = sb("stats", [NT, 7])

        meta_sem = st.enter_context(nc.semaphore("meta_sem"))
        obj_sem = st.enter_context(nc.semaphore("obj_sem"))
        act_sem = st.enter_context(nc.semaphore("act_sem"))
        dve_sem = st.enter_context(nc.semaphore("dve_sem"))
        st_sem = st.enter_context(nc.semaphore("st_sem"))
        block = st.enter_context(nc.Block())

        one_b = meta_t[:, ONE_:ONE_ + 1]

        # ACT landmarks
        A_E, A_LN, A_RX, A_L64, A_Q2, A_SP = 1, 2, 3, 5, 6, 7
        # DVE landmarks
        D_ALL = 16

        @block.sync
        def _(sync):
            sync.dma_start(out=meta_t[:], in_=meta[:]).then_inc(meta_sem, 16)
            sync.dma_start(out=obj_t[:], in_=obj[:]).then_inc(obj_sem, 16)
            sync.wait_ge(dve_sem, D_ALL)
            sync.wait_ge(act_sem, A_SP)
            sync.dma_start(out=part[:], in_=stats[:]).then_inc(st_sem, 16)

        @block.scalar
        def _(scalar):
            A = AF
            act = nc.scalar
            scalar.wait_ge(meta_sem, 16)
            act.activation(out=ebuf[:], in_=meta_t[:, O_:68],
                           func=A.Exp).then_inc(act_sem, 1)              # 1 A_E
            act.activation(out=sclb[:], in_=ebuf[:, 0:66], func=A.Ln,
                           bias=one_b).then_inc(act_sem, 1)              # 2 A_LN
            act.activation(out=rxb[:], in_=sclb[:, 1:66], func=A.Exp,
                           scale=-1.0).then_inc(act_sem, 1)              # 3 A_RX
            scalar.wait_ge(obj_sem, 16)
            act.activation(out=eob[:], in_=obj_t[:],
                           func=A.Exp).then_inc(act_sem, 1)              # 4
            act.activation(out=spb[:, C3:OBJW], in_=eob[:, C3:OBJW],
                           func=A.Ln, bias=one_b).then_inc(act_sem, 1)   # 5 A_L64
            scalar.wait_ge(dve_sem, 6)   # u1 written
            act.activation(out=q2[:], in_=u1[:],
                           func=A.Square).then_inc(act_sem, 1)           # 6 A_Q2
            act.activation(out=spb[:, 0:C3], in_=eob[:, 0:C3], func=A.Ln,
                           bias=one_b,
                           accum_out=stats[:, 4:5]).then_inc(act_sem, 1)  # 7 A_SP

        @block.vector
        def _(vector):
            vec = nc.vector
            vector.wait_ge(meta_sem, 16)
            vec.tensor_tensor(out=xy[:], in0=meta_t[:, Z_:Z_ + CC],
                              in1=meta_t[:, Y_:Y_ + CC],
                              op=ALU.mult).then_inc(dve_sem, 1)          # 1
            vector.wait_ge(act_sem, A_E)
            vec.tensor_scalar(out=dwh[:], in0=ebuf[:, 66:68], scalar1=EN4,
                              scalar2=E4, op0=ALU.max,
                              op1=ALU.min).then_inc(dve_sem, 1)          # 2
            vector.wait_ge(act_sem, A_LN)
            vec.tensor_tensor(out=bce[:], in0=sclb[:, 1:64], in1=xy[:],
                              op=ALU.subtract).then_inc(dve_sem, 1)      # 3
            vector.wait_ge(act_sem, A_RX)
            vec.scalar_tensor_tensor(out=pq[:, 0:2], in0=dwh[:], scalar=-0.5,
                                     in1=rxb[:, 63:65], op0=ALU.mult,
                                     op1=ALU.add).then_inc(dve_sem, 1)   # 4
            vec.scalar_tensor_tensor(out=pq[:, 2:4], in0=dwh[:], scalar=0.5,
                                     in1=rxb[:, 63:65], op0=ALU.mult,
                                     op1=ALU.add).then_inc(dve_sem, 1)   # 5
            vec.scalar_tensor_tensor(out=u1[:], in0=rxb[:, 0:CC], scalar=-1.0,
                                     in1=meta_t[:, Y_:Y_ + CC], op0=ALU.add,
                                     op1=ALU.add).then_inc(dve_sem, 1)   # 6
            nc.vector.drain()
            vec.tensor_tensor(out=dt_[:], in0=pq[:],
                              in1=meta_t[:, A_:A_ + 4],
                              op=ALU.add).then_inc(dve_sem, 1)           # 7
            vec.tensor_tensor(out=stats[:, 1:2], in0=sclb[:, 0:1],
                              in1=meta_t[:, O_:O_ + 1],
                              op=ALU.subtract).then_inc(dve_sem, 1)      # 8
            vec.tensor_tensor(out=stats[:, 3:4], in0=sclb[:, 0:1],
                              in1=meta_t[:, REC_:REC_ + 1],
                              op=ALU.mult).then_inc(dve_sem, 1)          # 9
            nc.vector.drain()
            vec.scalar_tensor_tensor(out=dab[:], in0=dt_[:], scalar=-1.0,
                                     in1=dt_[:], op0=ALU.mult,
                                     op1=ALU.max).then_inc(dve_sem, 1)   # 10
            nc.vector.drain()
            vec.tensor_scalar_min(out=mt[:], in0=dab[:],
                                  scalar1=1.0).then_inc(dve_sem, 1)      # 11
            vec.tensor_scalar(out=rl[:], in0=dab[:], scalar1=-1.0,
                              scalar2=0.0, op0=ALU.add,
                              op1=ALU.max).then_inc(dve_sem, 1)          # 12
            nc.vector.drain()
            vec.scalar_tensor_tensor(out=sl1o[:], in0=mt[:], scalar=0.125,
                                     in1=mt[:], op0=ALU.mult, op1=ALU.mult,
                                     accum_out=stats[:, 0:1]).then_inc(dve_sem, 1)  # 13
            vec.tensor_scalar(out=rlo[:], in0=rl[:], scalar1=0.25,
                              scalar2=0.0, op0=ALU.mult, op1=ALU.add,
                              accum_out=stats[:, 6:7]).then_inc(dve_sem, 1)  # 14
            vector.wait_ge(act_sem, A_L64)
            vec.reduce_sum(out=stats[:, 5:6], in_=spb[:, C3:OBJW],
                           axis=AX.X).then_inc(dve_sem, 1)               # 15
            vector.wait_ge(act_sem, A_Q2)
            vec.scalar_tensor_tensor(out=fqo[:], in0=q2[:], scalar=ALPHA / CC,
                                     in1=bce[:], op0=ALU.mult, op1=ALU.mult,
                                     accum_out=stats[:, 2:3]).then_inc(dve_sem, 1)  # 16 D_ALL

    return nc


def _get_bass():
    global _NC_CACHE
    if _NC_CACHE is None:
        _NC_CACHE = _build_bass()
    return _NC_CACHE


def _scale_rows(cls_p, reg_p, lt, hh, ww):
    """Per-core per-scale host prep: gather rows + pure-index metadata."""
    f = np.float32
    n = BL * T
    tx = lt[..., 1] * ww
    ty = lt[..., 2] * hh
    tw = lt[..., 3] * ww
    th = lt[..., 4] * hh
    gx = np.clip(tx, 0, ww - 1).astype(np.int32)
    gy = np.clip(ty, 0, hh - 1).astype(np.int32)
    bb = np.broadcast_to(np.arange(BL)[:, None], (BL, T))
    cl = cls_p[bb, :, gy, gx].reshape(n, 64)       # [n, 64] gathered cls
    rg = reg_p[bb, :, gy, gx].reshape(n, 4)        # [n, 4] gathered reg
    gxf = gx.astype(f)
    gyf = gy.astype(f)
    a = np.stack([gxf - tx + tw * 0.5, gyf - ty + th * 0.5,
                  gxf - tx - tw * 0.5, gyf - ty - th * 0.5], -1).reshape(n, 4)
    cids = lt[..., 0].astype(np.int32).reshape(n)
    y = (cids[:, None] == np.arange(CC)[None, :]).astype(f)
    cell = (bb * (hh * ww) + gy * ww + gx).reshape(n)
    uq, inv, cnts = np.unique(cell, return_inverse=True, return_counts=True)
    rec = (1.0 / cnts[inv]).astype(f)
    return cl, rg, a.astype(f), y, rec, len(uq)


def _prep_core_inputs(cls_p3, reg_p3, cls_p4, reg_p4, t3, t4):
    """Slice/gather full inputs into the 8 per-core input maps."""
    f = np.float32
    in_maps = []
    uniq3 = uniq4 = 0
    for c in range(M):
        sl = slice(c * BL, (c + 1) * BL)
        cl3, rg3, a3, y3, rec3, u3 = _scale_rows(
            cls_p3[sl], reg_p3[sl], t3[sl], H3, W3)
        cl4, rg4, a4, y4, rec4, u4 = _scale_rows(
            cls_p4[sl], reg_p4[sl], t4[sl], H4, W4)
        uniq3 += u3
        uniq4 += u4
        meta = np.zeros((NT, CM), f)
        for s, (cl, rg, a, y, rec) in enumerate(
                [(cl3, rg3, a3, y3, rec3), (cl4, rg4, a4, y4, rec4)]):
            rows = slice(s * BL * T, (s + 1) * BL * T)
            meta[rows, O_] = cl[:, 0]
            meta[rows, Z_:Z_ + CC] = cl[:, 1:]
            meta[rows, NR_:NR_ + 2] = -rg[:, 0:2]
            meta[rows, R23_:R23_ + 2] = rg[:, 2:4]
            meta[rows, Y_:Y_ + CC] = y
            meta[rows, A_:A_ + 4] = a
            meta[rows, REC_] = rec
            meta[rows, MK_ + s] = 1.0
        meta[:, ONE_] = 1.0
        obj = np.concatenate(
            [np.ascontiguousarray(cls_p3[sl, 0]).reshape(128, C3),
             np.ascontiguousarray(cls_p4[sl, 0]).reshape(128, OBJW - C3)],
            axis=1)
        in_maps.append({
            "obj": np.ascontiguousarray(obj).astype(ml_dtypes.bfloat16),
            "meta": meta,
        })
    return in_maps, uniq3, uniq4


def _combine(parts, uniq3, uniq4):
    """parts: [8, 128, 7] per-core per-target partials -> scalar loss.

    Rows 0:64 of each core are scale3 targets, 64:128 scale4 (float64
    combine: the per-core scalar all-reduce the device would otherwise do
    with a mask matmul)."""
    P = np.asarray(parts, np.float64)
    S3, S4 = P[:, 0:NT // 2, :].sum((0, 1)), P[:, NT // 2:, :].sum((0, 1))
    lb3, lb4 = S3[0] + S3[6], S4[0] + S4[6]
    lo3p, lo4p = S3[1], S4[1]
    lc3, lc4 = S3[2], S4[2]
    corr3, corr4 = S3[3], S4[3]
    sall3 = S3[4] + S4[4]                 # col4: scale3 softplus accum
    sall4 = S3[5] + S4[5]                 # col5: scale4 softplus reduce

    bg3 = (sall3 - corr3) / max(B * H3 * W3 - uniq3, 1.0)
    bg4 = (sall4 - corr4) / max(B * H4 * W4 - uniq4, 1.0)
    lo3 = lo3p + 0.05 * bg3
    lo4 = lo4p + 0.05 * bg4
    n = 2 * B * T
    lb = (lb3 + lb4) / n
    lc = (lc3 + lc4) / n
    lo = (lo3 + lo4) / max(n, 1)
    return np.float32(BBOX_W * lb + OBJ_W * lo + CLS_W * lc)


def kernel(cls_p3, reg_p3, cls_p4, reg_p4, t3, t4, _trace=False):
    f = np.float32
    in_maps, uniq3, uniq4 = _prep_core_inputs(
        np.asarray(cls_p3, f), np.asarray(reg_p3, f), np.asarray(cls_p4, f),
        np.asarray(reg_p4, f), np.asarray(t3, f), np.asarray(t4, f))
    nc = _get_bass()
    res = run_bass_kernel_spmd(nc, in_maps, core_ids=list(range(M)),
                               trace=_trace)
    parts = np.stack([r["part"] for r in res.results])
    out = _combine(parts, uniq3, uniq4)
    if _trace:
        return out, res
    return out


if __name__ == "__main__":
    rng = np.random.default_rng(0)
    inputs = {
        "cls_p3": rng.standard_normal((B, 64, H3, W3)).astype(np.float32),
        "reg_p3": rng.standard_normal((B, 4, H3, W3)).astype(np.float32),
        "cls_p4": rng.standard_normal((B, 64, H4, W4)).astype(np.float32),
        "reg_p4": rng.standard_normal((B, 4, H4, W4)).astype(np.float32),
        "t3": rng.random((B, T, 5), dtype=np.float32),
        "t4": rng.random((B, T, 5), dtype=np.float32),
    }
    print(kernel(**inputs))


# revision 24
# speedup vs baseline: 1.0781x; 1.0781x over previous
"""Trainium2 Bass kernel for nn_MCUDetectionLoss.

Strategy (data-parallel over batch, 8 cores, B=16 -> 2 images/core):

The loss touches (a) the objectness channel cls_p[:, 0] in full and (b) 64
gathered cells per image (obj + 63-class column + 4 reg values).  The host
ships each core:
  - obj  [128, 320]  objectness maps (scale3 flat 32768 = cols 0:256,
                     scale4 flat 8192 = cols 256:320)
  - meta [128, 140]  one row per target: gathered prediction values at the
                     target cell plus pure-index metadata (one-hot class,
                     box-offset constants, duplicate-cell weights, masks)

Device program per core: softplus of the gathered logits via one exp/ln
pass (ACT table set 6: exp+ln+copy, one table load), sigmoids via DVE
reciprocal of 1+e^x, smooth-L1 box loss, positive BCE, focal loss,
softplus-sum of the full obj map, and a [128,2]x[128,6] matmul reducing
everything to per-scale partial sums.  The host combines the 8 [2,6]
partials into the scalar.

Identities used (bce = BCEWithLogits):
  bce(x, 0) = softplus(x);  bce(x, 1) = softplus(x) - x
  sigmoid(x) = 1/(1 + exp(-x));  1 - sigmoid(x) = 1/(1 + exp(x))
  focal (1-pt)^2 = (y-p)^2
  exp(clip(x,-4,4)) = clip(exp(x), e^-4, e^4)
  sum softplus(obj)*bg = sum_all - sum_targets softplus(obj_t)/cnt_t
  smooth_l1(d) = mt*(|d| - mt/2),  mt = min(|d|, 1)
where cnt_t = multiplicity of the target's (image, cell) -- precomputed on
host from the integer cell indices (pure metadata, no tensor values).
"""

import math
import sys

for _p in ("/opt/trn_rl_repo", "/root/.axon_site/_ro/trn_rl_repo"):
    if _p not in sys.path:
        sys.path.append(_p)

import ml_dtypes
import numpy as np

import concourse.bass as bass
from concourse import mybir
from concourse.bass_utils import run_bass_kernel_spmd

AF = mybir.ActivationFunctionType
ALU = mybir.AluOpType
AX = mybir.AxisListType
F32 = mybir.dt.float32
BF16 = mybir.dt.bfloat16

ALPHA = 0.25
BBOX_W, OBJ_W, CLS_W = 2.0, 1.0, 0.5

M = 8          # cores
B, T, CC = 16, 32, 63
H3 = W3 = 128
H4 = W4 = 64
BL = B // M    # images per core
NT = 2 * BL * T     # 128 targets per core (rows 0:64 scale3, 64:128 scale4)
OBJW = (BL * H3 * W3 + BL * H4 * W4) // 128   # 320
C3 = BL * H3 * W3 // 128                      # 256 obj cols of scale3

# meta column layout
O_, Z_, NR_, R23_, Y_ = 0, 1, 64, 66, 68
A_, REC_, MK_, ONE_ = 131, 135, 136, 138
CM = 139

E4 = float(math.exp(4.0))
EN4 = float(math.exp(-4.0))

_NC_CACHE = None


def _build_bass():
    nc = bass.Bass("TRN2", target_bir_lowering=False, debug=False, num_devices=M)
    obj = nc.declare_dram_parameter("obj", [128, OBJW], BF16, isOutput=False)
    meta = nc.declare_dram_parameter("meta", [NT, CM], F32, isOutput=False)
    part = nc.declare_dram_parameter("part", [NT, 7], F32, isOutput=True)

    from contextlib import ExitStack
    with ExitStack() as st:
        def sb(name, shape, dt=F32):
            return st.enter_context(nc.sbuf_tensor(name, shape, dt))

        meta_t = sb("meta_t", [NT, CM]); obj_t = sb("obj_t", [128, OBJW], BF16)
        warm = sb("warm", [128, 1])
        ebuf = sb("ebuf", [NT, 68]); sclb = sb("sclb", [NT, 66])
        rxb = sb("rxb", [NT, 65])          # 0:63 = 1-p(z), 63:65 = sig(r01)
        eob = sb("eob", [128, OBJW]); spb = sb("spb", [128, OBJW])
        dwh = sb("dwh", [NT, 2])
        pq = sb("pq", [NT, 4]); dt_ = sb("dt_", [NT, 4]); dab = sb("dab", [NT, 4])
        mt = sb("mt", [NT, 4]); rl = sb("rl", [NT, 4])
        sl1o = sb("sl1o", [NT, 4]); rlo = sb("rlo", [NT, 4])
        xy = sb("xy", [NT, CC]); u1 = sb("u1", [NT, CC])
        bce = sb("bce", [NT, CC]); q2 = sb("q2", [NT, CC]); fqo = sb("fqo", [NT, CC])
        stats = sb("stats", [NT, 7])

        meta_sem = st.enter_context(nc.semaphore("meta_sem"))
        obj_sem = st.enter_context(nc.semaphore("obj_sem"))
        act_sem = st.enter_context(nc.semaphore("act_sem"))
        dve_sem = st.enter_context(nc.semaphore("dve_sem"))
        st_sem = st.enter_context(nc.semaphore("st_sem"))
        block = st.enter_context(nc.Block())

        one_b = meta_t[:, ONE_:ONE_ + 1]

        # ACT landmarks
        A_E, A_LN, A_RX, A_L64, A_Q2, A_SP = 2, 3, 4, 6, 7, 8
        # DVE landmarks
        D_ALL = 16

        @block.sync
        def _(sync):
            sync.dma_start(out=meta_t[:], in_=meta[:]).then_inc(meta_sem, 16)
            sync.dma_start(out=obj_t[:], in_=obj[:]).then_inc(obj_sem, 16)
            sync.wait_ge(dve_sem, D_ALL)
            sync.wait_ge(act_sem, A_SP)
            sync.dma_start(out=part[:], in_=stats[:]).then_inc(st_sem, 16)

        @block.scalar
        def _(scalar):
            A = AF
            act = nc.scalar
            # warmup: pulls the ACT table load to block start, overlapping
            # the input DMA wait (without it the load pins to the first
            # waiting activation and stalls the chain by ~1.2us)
            act.activation(out=warm[:], in_=warm[:],
                           func=A.Exp).then_inc(act_sem, 1)              # 1
            scalar.wait_ge(meta_sem, 16)
            act.activation(out=ebuf[:], in_=meta_t[:, O_:68],
                           func=A.Exp).then_inc(act_sem, 1)              # 2 A_E
            act.activation(out=sclb[:], in_=ebuf[:, 0:66], func=A.Ln,
                           bias=one_b).then_inc(act_sem, 1)              # 3 A_LN
            act.activation(out=rxb[:], in_=sclb[:, 1:66], func=A.Exp,
                           scale=-1.0).then_inc(act_sem, 1)              # 4 A_RX
            scalar.wait_ge(obj_sem, 16)
            act.activation(out=eob[:], in_=obj_t[:],
                           func=A.Exp).then_inc(act_sem, 1)              # 5
            act.activation(out=spb[:, C3:OBJW], in_=eob[:, C3:OBJW],
                           func=A.Ln, bias=one_b).then_inc(act_sem, 1)   # 6 A_L64
            scalar.wait_ge(dve_sem, 6)   # u1 written
            act.activation(out=q2[:], in_=u1[:],
                           func=A.Square).then_inc(act_sem, 1)           # 7 A_Q2
            act.activation(out=spb[:, 0:C3], in_=eob[:, 0:C3], func=A.Ln,
                           bias=one_b,
                           accum_out=stats[:, 4:5]).then_inc(act_sem, 1)  # 8 A_SP

        @block.vector
        def _(vector):
            vec = nc.vector
            vector.wait_ge(meta_sem, 16)
            vec.tensor_tensor(out=xy[:], in0=meta_t[:, Z_:Z_ + CC],
                              in1=meta_t[:, Y_:Y_ + CC],
                              op=ALU.mult).then_inc(dve_sem, 1)          # 1
            vector.wait_ge(act_sem, A_E)
            vec.tensor_scalar(out=dwh[:], in0=ebuf[:, 66:68], scalar1=EN4,
                              scalar2=E4, op0=ALU.max,
                              op1=ALU.min).then_inc(dve_sem, 1)          # 2
            vector.wait_ge(act_sem, A_LN)
            vec.tensor_tensor(out=bce[:], in0=sclb[:, 1:64], in1=xy[:],
                              op=ALU.subtract).then_inc(dve_sem, 1)      # 3
            vector.wait_ge(act_sem, A_RX)
            vec.scalar_tensor_tensor(out=pq[:, 0:2], in0=dwh[:], scalar=-0.5,
                                     in1=rxb[:, 63:65], op0=ALU.mult,
                                     op1=ALU.add).then_inc(dve_sem, 1)   # 4
            vec.scalar_tensor_tensor(out=pq[:, 2:4], in0=dwh[:], scalar=0.5,
                                     in1=rxb[:, 63:65], op0=ALU.mult,
                                     op1=ALU.add).then_inc(dve_sem, 1)   # 5
            vec.scalar_tensor_tensor(out=u1[:], in0=rxb[:, 0:CC], scalar=-1.0,
                                     in1=meta_t[:, Y_:Y_ + CC], op0=ALU.add,
                                     op1=ALU.add).then_inc(dve_sem, 1)   # 6
            nc.vector.drain()
            vec.tensor_tensor(out=dt_[:], in0=pq[:],
                              in1=meta_t[:, A_:A_ + 4],
                              op=ALU.add).then_inc(dve_sem, 1)           # 7
            vec.tensor_tensor(out=stats[:, 1:2], in0=sclb[:, 0:1],
                              in1=meta_t[:, O_:O_ + 1],
                              op=ALU.subtract).then_inc(dve_sem, 1)      # 8
            vec.tensor_tensor(out=stats[:, 3:4], in0=sclb[:, 0:1],
                              in1=meta_t[:, REC_:REC_ + 1],
                              op=ALU.mult).then_inc(dve_sem, 1)          # 9
            nc.vector.drain()
            vec.scalar_tensor_tensor(out=dab[:], in0=dt_[:], scalar=-1.0,
                                     in1=dt_[:], op0=ALU.mult,
                                     op1=ALU.max).then_inc(dve_sem, 1)   # 10
            nc.vector.drain()
            vec.tensor_scalar_min(out=mt[:], in0=dab[:],
                                  scalar1=1.0).then_inc(dve_sem, 1)      # 11
            vec.tensor_scalar(out=rl[:], in0=dab[:], scalar1=-1.0,
                              scalar2=0.0, op0=ALU.add,
                              op1=ALU.max).then_inc(dve_sem, 1)          # 12
            vector.wait_ge(act_sem, A_L64)
            vec.reduce_sum(out=stats[:, 5:6], in_=spb[:, C3:OBJW],
                           axis=AX.X).then_inc(dve_sem, 1)               # 13
            nc.vector.drain()
            vec.scalar_tensor_tensor(out=sl1o[:], in0=mt[:], scalar=0.125,
                                     in1=mt[:], op0=ALU.mult, op1=ALU.mult,
                                     accum_out=stats[:, 0:1]).then_inc(dve_sem, 1)  # 14
            vec.tensor_scalar(out=rlo[:], in0=rl[:], scalar1=0.25,
                              scalar2=0.0, op0=ALU.mult, op1=ALU.add,
                              accum_out=stats[:, 6:7]).then_inc(dve_sem, 1)  # 15
            vector.wait_ge(act_sem, A_Q2)
            vec.scalar_tensor_tensor(out=fqo[:], in0=q2[:], scalar=ALPHA / CC,
                                     in1=bce[:], op0=ALU.mult, op1=ALU.mult,
                                     accum_out=stats[:, 2:3]).then_inc(dve_sem, 1)  # 16 D_ALL

    return nc


def _get_bass():
    global _NC_CACHE
    if _NC_CACHE is None:
        _NC_CACHE = _build_bass()
    return _NC_CACHE


def _scale_rows(cls_p, reg_p, lt, hh, ww):
    """Per-core per-scale host prep: gather rows + pure-index metadata."""
    f = np.float32
    n = BL * T
    tx = lt[..., 1] * ww
    ty = lt[..., 2] * hh
    tw = lt[..., 3] * ww
    th = lt[..., 4] * hh
    gx = np.clip(tx, 0, ww - 1).astype(np.int32)
    gy = np.clip(ty, 0, hh - 1).astype(np.int32)
    bb = np.broadcast_to(np.arange(BL)[:, None], (BL, T))
    cl = cls_p[bb, :, gy, gx].reshape(n, 64)       # [n, 64] gathered cls
    rg = reg_p[bb, :, gy, gx].reshape(n, 4)        # [n, 4] gathered reg
    gxf = gx.astype(f)
    gyf = gy.astype(f)
    a = np.stack([gxf - tx + tw * 0.5, gyf - ty + th * 0.5,
                  gxf - tx - tw * 0.5, gyf - ty - th * 0.5], -1).reshape(n, 4)
    cids = lt[..., 0].astype(np.int32).reshape(n)
    y = (cids[:, None] == np.arange(CC)[None, :]).astype(f)
    cell = (bb * (hh * ww) + gy * ww + gx).reshape(n)
    uq, inv, cnts = np.unique(cell, return_inverse=True, return_counts=True)
    rec = (1.0 / cnts[inv]).astype(f)
    return cl, rg, a.astype(f), y, rec, len(uq)


def _prep_core_inputs(cls_p3, reg_p3, cls_p4, reg_p4, t3, t4):
    """Slice/gather full inputs into the 8 per-core input maps."""
    f = np.float32
    in_maps = []
    uniq3 = uniq4 = 0
    for c in range(M):
        sl = slice(c * BL, (c + 1) * BL)
        cl3, rg3, a3, y3, rec3, u3 = _scale_rows(
            cls_p3[sl], reg_p3[sl], t3[sl], H3, W3)
        cl4, rg4, a4, y4, rec4, u4 = _scale_rows(
            cls_p4[sl], reg_p4[sl], t4[sl], H4, W4)
        uniq3 += u3
        uniq4 += u4
        meta = np.zeros((NT, CM), f)
        for s, (cl, rg, a, y, rec) in enumerate(
                [(cl3, rg3, a3, y3, rec3), (cl4, rg4, a4, y4, rec4)]):
            rows = slice(s * BL * T, (s + 1) * BL * T)
            meta[rows, O_] = cl[:, 0]
            meta[rows, Z_:Z_ + CC] = cl[:, 1:]
            meta[rows, NR_:NR_ + 2] = -rg[:, 0:2]
            meta[rows, R23_:R23_ + 2] = rg[:, 2:4]
            meta[rows, Y_:Y_ + CC] = y
            meta[rows, A_:A_ + 4] = a
            meta[rows, REC_] = rec
            meta[rows, MK_ + s] = 1.0
        meta[:, ONE_] = 1.0
        obj = np.concatenate(
            [np.ascontiguousarray(cls_p3[sl, 0]).reshape(128, C3),
             np.ascontiguousarray(cls_p4[sl, 0]).reshape(128, OBJW - C3)],
            axis=1)
        in_maps.append({
            "obj": np.ascontiguousarray(obj).astype(ml_dtypes.bfloat16),
            "meta": meta,
        })
    return in_maps, uniq3, uniq4


def _combine(parts, uniq3, uniq4):
    """parts: [8, 128, 7] per-core per-target partials -> scalar loss.

    Rows 0:64 of each core are scale3 targets, 64:128 scale4 (float64
    combine: the per-core scalar all-reduce the device would otherwise do
    with a mask matmul)."""
    P = np.asarray(parts, np.float64)
    S3, S4 = P[:, 0:NT // 2, :].sum((0, 1)), P[:, NT // 2:, :].sum((0, 1))
    lb3, lb4 = S3[0] + S3[6], S4[0] + S4[6]
    lo3p, lo4p = S3[1], S4[1]
    lc3, lc4 = S3[2], S4[2]
    corr3, corr4 = S3[3], S4[3]
    sall3 = S3[4] + S4[4]                 # col4: scale3 softplus accum
    sall4 = S3[5] + S4[5]                 # col5: scale4 softplus reduce

    bg3 = (sall3 - corr3) / max(B * H3 * W3 - uniq3, 1.0)
    bg4 = (sall4 - corr4) / max(B * H4 * W4 - uniq4, 1.0)
    lo3 = lo3p + 0.05 * bg3
    lo4 = lo4p + 0.05 * bg4
    n = 2 * B * T
    lb = (lb3 + lb4) / n
    lc = (lc3 + lc4) / n
    lo = (lo3 + lo4) / max(n, 1)
    return np.float32(BBOX_W * lb + OBJ_W * lo + CLS_W * lc)


def kernel(cls_p3, reg_p3, cls_p4, reg_p4, t3, t4, _trace=False):
    f = np.float32
    in_maps, uniq3, uniq4 = _prep_core_inputs(
        np.asarray(cls_p3, f), np.asarray(reg_p3, f), np.asarray(cls_p4, f),
        np.asarray(reg_p4, f), np.asarray(t3, f), np.asarray(t4, f))
    nc = _get_bass()
    res = run_bass_kernel_spmd(nc, in_maps, core_ids=list(range(M)),
                               trace=_trace)
    parts = np.stack([r["part"] for r in res.results])
    out = _combine(parts, uniq3, uniq4)
    if _trace:
        return out, res
    return out


if __name__ == "__main__":
    rng = np.random.default_rng(0)
    inputs = {
        "cls_p3": rng.standard_normal((B, 64, H3, W3)).astype(np.float32),
        "reg_p3": rng.standard_normal((B, 4, H3, W3)).astype(np.float32),
        "cls_p4": rng.standard_normal((B, 64, H4, W4)).astype(np.float32),
        "reg_p4": rng.standard_normal((B, 4, H4, W4)).astype(np.float32),
        "t3": rng.random((B, T, 5), dtype=np.float32),
        "t4": rng.random((B, T, 5), dtype=np.float32),
    }
    print(kernel(**inputs))
